# revision 17
# baseline (speedup 1.0000x reference)
"""Trainium2 Bass kernel for nn_AttentionBlock (B=4, C=128, T=4096, K=64, V=128).

Sharding: 8 cores = 4 batches x 2 j-groups. Core (b, g) owns global j-tiles
{2k+g : k=0..15} of batch b. Softmax runs over the query axis i (local to a
j column), so a j-split is embarrassingly parallel up to a final sum of the
partial read matrices, done on the host.

Per local j-tile k the live i region (i <= j) is covered by a uniform strip
i in [0, (2k+2)*128): the ceil over both j-groups, so one SPMD program fits
all cores; a per-core additive mask on the strip's last 256 columns encodes
both the causal triangle and the g=0 overhang (mask content is data, so it
may differ per core while the program stays uniform).

Bias trick: softmax runs over i, so any per-column (per-j) additive term in
the logits cancels — bq drops out entirely. The remaining bk contribution
q_i . bk is an extra rank-1 row in the contraction: the host augments Wq
with column Wq @ bk (so the projection computes that row for free) and the
K^T side gets a constant ones row.

Logits run as fp8e4 DoubleRow matmuls: contraction 66 = 33 partitions x 2
k-subtiles (64 K dims + ones row + zero pad), 2x PE throughput. e / vs and
the read matmuls stay bf16.

Device computes outT = partial read^T [V=128, T] (rows :C of the final
output are x itself and are stitched on the host, which also sums the two
j-group partials per batch).
"""

import numpy as np

_B, _C, _T = 4, 128, 4096
_K, _V = 64, 128
_JT = 16           # local j-tiles per core (128 wide) -> 2048 local j columns
_LG = 1536         # logits PSUM chunk width (3 banks)
_NEG = -1.0e30

# read passes: (i0, i1, kmin) — pass covers out cols [i0, i1), summing
# strips k >= kmin (the strips whose width exceeds i0). The burst for a
# pass is emitted during strip kmin-1 (vs[kmin] exists by then); the final
# (0,256) pass is split so only its k'=0 matmul trails the last exp.
_PASSES = [(3584, 4096, 14), (3072, 3584, 12), (2560, 3072, 10),
           (2048, 2560, 8), (1536, 2048, 6), (1024, 1536, 4),
           (512, 1024, 2), (256, 512, 1), (0, 256, 0)]


def _W(k):
    return (2 * k + 2) * 128


def _S(k):
    return 128 * k * (k + 1)


_ETOT = _S(_JT)    # 34816 columns of e per core

_cache = {}


def _build_nc():
    from contextlib import ExitStack

    import concourse.tile as tile
    from concourse import bacc, mybir
    from concourse.masks import make_identity

    f32 = mybir.dt.float32
    f32r = mybir.dt.float32r
    bf16 = mybir.dt.bfloat16
    f8 = mybir.dt.float8e4
    AF = mybir.ActivationFunctionType
    DR = mybir.MatmulPerfMode.DoubleRow

    nc = bacc.Bacc("TRN2", target_bir_lowering=False)

    xb_d = nc.dram_tensor("xb", [_C, _T], f32r, kind="ExternalInput")
    xj_d = nc.dram_tensor("xj", [_C, _JT * 128], f32r, kind="ExternalInput")
    # packed constants: one DMA issue each (SP sequencer issue is ~650ns
    # per dma_start, so merged loads shorten the pipeline head)
    cr_d = nc.dram_tensor("cr", [_C, 384], f32r, kind="ExternalInput")
    cf_d = nc.dram_tensor("cf", [_C, 386], f32, kind="ExternalInput")
    out_d = nc.dram_tensor("out", [_V, _T], f32, kind="ExternalOutput")

    with tile.TileContext(nc) as tc, ExitStack() as ctx:
        singles = ctx.enter_context(tc.tile_pool(name="singles", bufs=1))
        work = ctx.enter_context(tc.tile_pool(name="work", bufs=2))
        small = ctx.enter_context(tc.tile_pool(name="small", bufs=8))
        lg = ctx.enter_context(tc.tile_pool(name="lg", bufs=2, space="PSUM"))
        rd = ctx.enter_context(tc.tile_pool(name="rd", bufs=1, space="PSUM"))
        vv = ctx.enter_context(tc.tile_pool(name="vv", bufs=1, space="PSUM"))

        # ---------------- input DMA (ordered for pipeline head) ---------
        xb_f = singles.tile([_C, _T], f32r)
        xj_f = singles.tile([_C, _JT * 128], f32r)

        c_r = singles.tile([_C, 384], f32r)
        nc.sync.dma_start(out=c_r, in_=cr_d[:])
        nc.sync.dma_start(out=xj_f[:, 1536:2048], in_=xj_d[:, 1536:2048])
        nc.sync.dma_start(out=xb_f[:, 3072:4096], in_=xb_d[:, 3072:4096])
        c_f = singles.tile([_C, 386], f32)
        nc.scalar.dma_start(out=c_f, in_=cf_d[:])
        nc.gpsimd.dma_start(out=xb_f[:, 0:3072], in_=xb_d[:, 0:3072])
        nc.gpsimd.dma_start(out=xj_f[:, 0:1536], in_=xj_d[:, 0:1536])

        wq_s = c_r[:, 0:_K]
        wk_s = c_r[:, _K:2 * _K]
        wv_p = c_r[:, 128:384]
        bv_r = c_f[:, 0:128]
        mask_f = c_f[:, 128:384]
        bq_s = c_f[0:_K, 384:385]
        bk_s = c_f[0:_K, 385:386]

        mask_bf = singles.tile([128, 256], bf16)
        nc.vector.tensor_copy(mask_bf, mask_f)
        id_bf = singles.tile([128, 128], bf16)
        make_identity(nc, id_bf[:])

        # ------------- Q^T / K^T projections (borrow lg pool) -----------
        qt_bf = singles.tile([_K, _T], bf16)
        kt_bf = singles.tile([_K, _JT * 128], bf16)

        def proj(dst, w_s, src_f, b_s, a):
            # one 512-col f32r chunk (one PSUM bank) + DVE bias add
            b = a + 512
            ps = lg.tile([128, _LG], f32, tag="lg")
            nc.tensor.matmul(ps[0:_K, 0:512], w_s, src_f[:, a:b],
                             start=True, stop=True)
            nc.vector.tensor_scalar_add(dst[:, a:b], ps[0:_K, 0:512],
                                        b_s)

        proj(kt_bf, wk_s, xj_f, bk_s, 1536)   # K^T tiles 12..15

        # ---------------- attention ----------------
        e_all = singles.tile([128, _ETOT], bf16)
        vs_bf = singles.tile([128, _JT, _V], bf16)

        def logits_chunk(k, c, n_k):
            W, S = _W(k), _S(k)
            a, b = c * _LG, min((c + 1) * _LG, W)
            ps = lg.tile([128, _LG], f32, tag="lg")
            diag = c == n_k - 1
            m0 = (b - a) - 256
            kt_k = kt_bf[:, k * 128:(k + 1) * 128]
            for o in range(0, b - a, 512):
                w = min(o + 512, b - a) - o
                last_bank = diag and (o + w == b - a)
                nc.tensor.matmul(ps[:, o:o + w], kt_k,
                                 qt_bf[:, a + o:a + o + w],
                                 start=True, stop=not last_bank)
            if diag:
                nc.tensor.matmul(ps[:, m0:m0 + 256], id_bf, mask_bf,
                                 start=False, stop=True)
            acc = small.tile([128, 1], f32, tag="acc")
            nc.scalar.activation(out=e_all[:, S + a:S + b],
                                 in_=ps[:, 0:b - a],
                                 func=AF.Exp, scale=0.125, accum_out=acc)
            return acc

        def strip_tail(k, accs):
            # V projection (1-bank PSUM); bv add + 1/s scale on DVE
            v_ps = vv.tile([128, 256], f32, tag="vv")
            nc.tensor.matmul(v_ps, xj_f[:, k * 128:(k + 1) * 128], wv_p,
                             start=True, stop=True)
            vb = work.tile([128, _V], f32, tag="vb")
            nc.vector.tensor_add(vb, v_ps[:, 0:_V], bv_r)
            s_t = accs[0]
            for extra in accs[1:]:
                s2 = small.tile([128, 1], f32, tag="s")
                nc.vector.tensor_add(s2, s_t, extra)
                s_t = s2
            rs = small.tile([128, 1], f32, tag="rs")
            nc.vector.reciprocal(rs, s_t)
            nc.vector.tensor_scalar_mul(vs_bf[:, k, :], vb, rs)

        def read_pass(i0, i1, kmin, klo=None):
            # accumulate read^T over strips kmin..15 for out cols [i0, i1)
            ps_r = rd.tile([128, 512], f32, tag="rd")
            for kk in range(_JT - 1, (klo if klo is not None else kmin) - 1,
                            -1):
                w = min(_W(kk), i1) - i0
                nc.tensor.matmul(ps_r[0:_V, 0:w], vs_bf[:, kk, :],
                                 e_all[:, _S(kk) + i0:_S(kk) + i0 + w],
                                 start=(kk == _JT - 1), stop=(kk == kmin))
            return ps_r

        def drain_pass(ps_r, i0, i1):
            ot = work.tile([_V, 512], f32, tag="osb")
            nc.vector.tensor_copy(ot[:, 0:i1 - i0], ps_r[0:_V, 0:i1 - i0])
            nc.sync.dma_start(out=out_d[:, i0:i1], in_=ot[:, 0:i1 - i0])

        # strip 15 interleaved with the remaining projections (head)
        proj(qt_bf, wq_s, xb_f, bq_s, 3072)   # qt hi
        proj(qt_bf, wq_s, xb_f, bq_s, 3584)
        accs15 = [logits_chunk(15, 2, 3)]
        for a in (0, 512, 1024):
            proj(qt_bf, wq_s, xb_f, bq_s, a)
        accs15.append(logits_chunk(15, 0, 3))
        for a in (1536, 2048, 2560):
            proj(qt_bf, wq_s, xb_f, bq_s, a)
        accs15.append(logits_chunk(15, 1, 3))
        for a in (0, 512, 1024):              # K^T tiles 0..11
            proj(kt_bf, wk_s, xj_f, bk_s, a)
        strip_tail(15, accs15)

        pending = []
        for k in range(_JT - 2, -1, -1):
            n_k = -(-_W(k) // _LG)
            accs = [logits_chunk(k, c, n_k)
                    for c in [n_k - 1] + list(range(n_k - 1))]

            # read burst deferred from the previous strip boundary: PE
            # works on it while ACT chews this strip's exps
            for (i0, i1, kmin) in pending:
                drain_pass(read_pass(i0, i1, kmin), i0, i1)
            pending = [p for p in _PASSES if p[2] == k]

            strip_tail(k, accs)

        # tail: the final (0,256) pass — only k'=0 remains after vs[0]
        (i0, i1, _) = _PASSES[-1]
        ps_last = read_pass(i0, i1, 0, klo=1)
        nc.tensor.matmul(ps_last[0:_V, 0:256], vs_bf[:, 0, :],
                         e_all[:, _S(0) + i0:_S(0) + i0 + 256],
                         start=False, stop=True)
        drain_pass(ps_last, i0, i1)

    nc.compile()
    return nc


def _get_nc():
    if "nc" not in _cache:
        _cache["nc"] = _build_nc()
    return _cache["nc"]


def _masks(g):
    """Additive mask for the last 256 columns of every strip.

    Strip for local tile k covers i in [0, (2k+2)*128); its last 256
    columns are i = 2k*128 + u, u in [0, 256). Partition p holds global
    j = (2k+g)*128 + p, so live (i <= j) iff u <= 128*g + p.
    """
    m = np.zeros((128, 256), np.float32)
    p = np.arange(128)[:, None]
    u = np.arange(256)[None, :]
    m[:] = np.where(u <= 128 * g + p, 0.0, _NEG)
    return m


def kernel(**inputs):
    from concourse.bass_utils import run_bass_kernel_spmd

    x = np.ascontiguousarray(np.asarray(inputs["x"], dtype=np.float32))
    Wq = np.ascontiguousarray(np.asarray(inputs["Wq"], dtype=np.float32))
    Wk = np.ascontiguousarray(np.asarray(inputs["Wk"], dtype=np.float32))
    Wv = np.asarray(inputs["Wv"], dtype=np.float32)
    bq = np.ascontiguousarray(
        np.asarray(inputs["bq"], dtype=np.float32).reshape(_K, 1))
    bk = np.ascontiguousarray(
        np.asarray(inputs["bk"], dtype=np.float32).reshape(_K, 1))
    bv = np.asarray(inputs["bv"], dtype=np.float32).ravel()

    c_r = np.zeros((_C, 384), np.float32)
    c_r[:, :_K] = Wq
    c_r[:, _K:2 * _K] = Wk
    c_r[:, 128:128 + _V] = Wv

    nc = _get_nc()
    in_maps = []
    for core in range(8):
        b, g = divmod(core, 2)
        # this core's j columns: global tiles {2k+g}, i.e. starts 256k+128g
        cols = ((np.arange(_JT) * 256 + 128 * g)[:, None]
                + np.arange(128)[None, :]).ravel()
        c_f = np.zeros((_C, 386), np.float32)
        c_f[:, 0:128] = bv[None, :]
        c_f[:, 128:384] = _masks(g)
        c_f[0:_K, 384] = bq.ravel()
        c_f[0:_K, 385] = bk.ravel()
        in_maps.append({
            "xb": np.ascontiguousarray(x[b]),
            "xj": np.ascontiguousarray(x[b][:, cols]),
            "cr": c_r, "cf": np.ascontiguousarray(c_f),
        })

    trace = bool(_cache.get("trace"))
    res = run_bass_kernel_spmd(nc, in_maps, core_ids=list(range(8)),
                               trace=trace)
    _cache["last_result"] = res

    parts = [r["out"] for r in res.results]
    out = np.empty((_B, _C + _V, _T), np.float32)
    for b in range(_B):
        out[b, :_C] = x[b]
        out[b, _C:] = parts[2 * b] + parts[2 * b + 1]
    return out


# revision 18
# speedup vs baseline: 1.0763x; 1.0763x over previous
"""Trainium2 Bass kernel for nn_AttentionBlock (B=4, C=128, T=4096, K=64, V=128).

Sharding: 8 cores = 4 batches x 2 j-groups. Core (b, g) owns global j-tiles
{2k+g : k=0..15} of batch b. Softmax runs over the query axis i (local to a
j column), so a j-split is embarrassingly parallel up to a final sum of the
partial read matrices, done on the host.

Per local j-tile k the live i region (i <= j) is covered by a uniform strip
i in [0, (2k+2)*128): the ceil over both j-groups, so one SPMD program fits
all cores; a per-core additive mask on the strip's last 256 columns encodes
both the causal triangle and the g=0 overhang (mask content is data, so it
may differ per core while the program stays uniform).

Bias trick: softmax runs over i, so any per-column (per-j) additive term in
the logits cancels — bq drops out entirely. The remaining bk contribution
q_i . bk is an extra rank-1 row in the contraction: the host augments Wq
with column Wq @ bk (so the projection computes that row for free) and the
K^T side gets a constant ones row.

Logits run as fp8e4 DoubleRow matmuls: contraction 66 = 33 partitions x 2
k-subtiles (64 K dims + ones row + zero pad), 2x PE throughput. e / vs and
the read matmuls stay bf16.

Device computes outT = partial read^T [V=128, T] (rows :C of the final
output are x itself and are stitched on the host, which also sums the two
j-group partials per batch).
"""

import numpy as np

_B, _C, _T = 4, 128, 4096
_K, _V = 64, 128
_JT = 16           # local j-tiles per core (128 wide) -> 2048 local j columns
_LG = 1536         # logits PSUM chunk width (3 banks)
_NEG = -1.0e30

# read passes: (i0, i1, kmin) — pass covers out cols [i0, i1), summing
# strips k >= kmin (the strips whose width exceeds i0). The burst for a
# pass is emitted during strip kmin-1 (vs[kmin] exists by then); the final
# (0,256) pass is split so only its k'=0 matmul trails the last exp.
_PASSES = [(3584, 4096, 14), (3072, 3584, 12), (2560, 3072, 10),
           (2048, 2560, 8), (1536, 2048, 6), (1024, 1536, 4),
           (512, 1024, 2), (256, 512, 1), (0, 256, 0)]


def _W(k):
    return (2 * k + 2) * 128


def _S(k):
    return 128 * k * (k + 1)


_ETOT = _S(_JT)    # 34816 columns of e per core

_cache = {}


def _build_nc():
    from contextlib import ExitStack

    import concourse.tile as tile
    from concourse import bacc, mybir
    from concourse.masks import make_identity

    f32 = mybir.dt.float32
    f32r = mybir.dt.float32r
    bf16 = mybir.dt.bfloat16
    f8 = mybir.dt.float8e4
    AF = mybir.ActivationFunctionType
    DR = mybir.MatmulPerfMode.DoubleRow

    nc = bacc.Bacc("TRN2", target_bir_lowering=False)

    xb_d = nc.dram_tensor("xb", [_C, _T], bf16, kind="ExternalInput")
    xj_d = nc.dram_tensor("xj", [_C, _JT * 128], bf16, kind="ExternalInput")
    # packed constants: one DMA issue each (SP sequencer issue is ~650ns
    # per dma_start, so merged loads shorten the pipeline head)
    cb_d = nc.dram_tensor("cb", [_C, 512], bf16, kind="ExternalInput")
    cf_d = nc.dram_tensor("cf", [_C, 130], f32, kind="ExternalInput")
    out_d = nc.dram_tensor("out", [_V, _T], f32, kind="ExternalOutput")

    with tile.TileContext(nc) as tc, ExitStack() as ctx:
        singles = ctx.enter_context(tc.tile_pool(name="singles", bufs=1))
        work = ctx.enter_context(tc.tile_pool(name="work", bufs=2))
        small = ctx.enter_context(tc.tile_pool(name="small", bufs=8))
        lg = ctx.enter_context(tc.tile_pool(name="lg", bufs=2, space="PSUM"))
        rd = ctx.enter_context(tc.tile_pool(name="rd", bufs=1, space="PSUM"))
        vv = ctx.enter_context(tc.tile_pool(name="vv", bufs=1, space="PSUM"))

        # ---------------- input DMA (ordered for pipeline head) ---------
        xb_f = singles.tile([_C, _T], bf16)
        xj_f = singles.tile([_C, _JT * 128], bf16)

        c_b = singles.tile([_C, 512], bf16)
        nc.sync.dma_start(out=c_b, in_=cb_d[:])
        nc.sync.dma_start(out=xj_f[:, 1536:2048], in_=xj_d[:, 1536:2048])
        nc.sync.dma_start(out=xb_f[:, 3072:4096], in_=xb_d[:, 3072:4096])
        c_f = singles.tile([_C, 130], f32)
        nc.scalar.dma_start(out=c_f, in_=cf_d[:])
        nc.gpsimd.dma_start(out=xb_f[:, 0:1536], in_=xb_d[:, 0:1536])
        nc.gpsimd.dma_start(out=xb_f[:, 1536:3072], in_=xb_d[:, 1536:3072])
        nc.gpsimd.dma_start(out=xj_f[:, 0:1536], in_=xj_d[:, 0:1536])

        wq_s = c_b[:, 0:_K]
        wk_s = c_b[:, _K:2 * _K]
        wv_p = c_b[:, 128:256]
        mask_bf = c_b[:, 256:512]
        bv_r = c_f[:, 0:128]
        bq_s = c_f[0:_K, 128:129]
        bk_s = c_f[0:_K, 129:130]

        id_bf = singles.tile([128, 128], bf16)
        make_identity(nc, id_bf[:])

        # ------------- Q^T / K^T projections (borrow lg pool) -----------
        qt_bf = singles.tile([_K, _T], bf16)
        kt_bf = singles.tile([_K, _JT * 128], bf16)

        def proj(dst, w_s, src_f, b_s, a):
            # one 512-col f32r chunk (one PSUM bank) + DVE bias add
            b = a + 512
            ps = lg.tile([128, _LG], f32, tag="lg")
            nc.tensor.matmul(ps[0:_K, 0:512], w_s, src_f[:, a:b],
                             start=True, stop=True)
            nc.vector.tensor_scalar_add(dst[:, a:b], ps[0:_K, 0:512],
                                        b_s)

        proj(kt_bf, wk_s, xj_f, bk_s, 1536)   # K^T tiles 12..15

        # ---------------- attention ----------------
        e_all = singles.tile([128, _ETOT], bf16)
        vs_bf = singles.tile([128, _JT, _V], bf16)

        def logits_chunk(k, c, n_k):
            W, S = _W(k), _S(k)
            a, b = c * _LG, min((c + 1) * _LG, W)
            ps = lg.tile([128, _LG], f32, tag="lg")
            diag = c == n_k - 1
            m0 = (b - a) - 256
            kt_k = kt_bf[:, k * 128:(k + 1) * 128]
            for o in range(0, b - a, 512):
                w = min(o + 512, b - a) - o
                last_bank = diag and (o + w == b - a)
                nc.tensor.matmul(ps[:, o:o + w], kt_k,
                                 qt_bf[:, a + o:a + o + w],
                                 start=True, stop=not last_bank)
            if diag:
                nc.tensor.matmul(ps[:, m0:m0 + 256], id_bf, mask_bf,
                                 start=False, stop=True)
            acc = small.tile([128, 1], f32, tag="acc")
            nc.scalar.activation(out=e_all[:, S + a:S + b],
                                 in_=ps[:, 0:b - a],
                                 func=AF.Exp, scale=0.125, accum_out=acc)
            return acc

        def strip_tail(k, accs):
            # V projection (1-bank PSUM); bv add + 1/s scale on DVE
            v_ps = vv.tile([128, _V], f32, tag="vv")
            nc.tensor.matmul(v_ps, xj_f[:, k * 128:(k + 1) * 128], wv_p,
                             start=True, stop=True)
            vb = work.tile([128, _V], f32, tag="vb")
            nc.vector.tensor_add(vb, v_ps, bv_r)
            s_t = accs[0]
            for extra in accs[1:]:
                s2 = small.tile([128, 1], f32, tag="s")
                nc.vector.tensor_add(s2, s_t, extra)
                s_t = s2
            rs = small.tile([128, 1], f32, tag="rs")
            nc.vector.reciprocal(rs, s_t)
            nc.vector.tensor_scalar_mul(vs_bf[:, k, :], vb, rs)

        def read_pass(i0, i1, kmin, klo=None):
            # accumulate read^T over strips kmin..15 for out cols [i0, i1)
            ps_r = rd.tile([128, 512], f32, tag="rd")
            for kk in range(_JT - 1, (klo if klo is not None else kmin) - 1,
                            -1):
                w = min(_W(kk), i1) - i0
                nc.tensor.matmul(ps_r[0:_V, 0:w], vs_bf[:, kk, :],
                                 e_all[:, _S(kk) + i0:_S(kk) + i0 + w],
                                 start=(kk == _JT - 1), stop=(kk == kmin))
            return ps_r

        def drain_pass(ps_r, i0, i1):
            ot = work.tile([_V, 512], f32, tag="osb")
            nc.vector.tensor_copy(ot[:, 0:i1 - i0], ps_r[0:_V, 0:i1 - i0])
            nc.sync.dma_start(out=out_d[:, i0:i1], in_=ot[:, 0:i1 - i0])

        # strip 15 interleaved with the remaining projections (head)
        proj(qt_bf, wq_s, xb_f, bq_s, 3072)   # qt hi
        proj(qt_bf, wq_s, xb_f, bq_s, 3584)
        accs15 = [logits_chunk(15, 2, 3)]
        for a in (0, 512, 1024):
            proj(qt_bf, wq_s, xb_f, bq_s, a)
        accs15.append(logits_chunk(15, 0, 3))
        for a in (1536, 2048, 2560):
            proj(qt_bf, wq_s, xb_f, bq_s, a)
        accs15.append(logits_chunk(15, 1, 3))
        for a in (0, 512, 1024):              # K^T tiles 0..11
            proj(kt_bf, wk_s, xj_f, bk_s, a)
        strip_tail(15, accs15)

        pending = []
        for k in range(_JT - 2, -1, -1):
            n_k = -(-_W(k) // _LG)
            accs = [logits_chunk(k, c, n_k)
                    for c in [n_k - 1] + list(range(n_k - 1))]

            # read burst deferred from the previous strip boundary: PE
            # works on it while ACT chews this strip's exps
            for (i0, i1, kmin) in pending:
                drain_pass(read_pass(i0, i1, kmin), i0, i1)
            pending = [p for p in _PASSES if p[2] == k]

            strip_tail(k, accs)

        # tail: the final (0,256) pass — only k'=0 remains after vs[0]
        (i0, i1, _) = _PASSES[-1]
        ps_last = read_pass(i0, i1, 0, klo=1)
        nc.tensor.matmul(ps_last[0:_V, 0:256], vs_bf[:, 0, :],
                         e_all[:, _S(0) + i0:_S(0) + i0 + 256],
                         start=False, stop=True)
        drain_pass(ps_last, i0, i1)

    nc.compile()
    return nc


def _get_nc():
    if "nc" not in _cache:
        _cache["nc"] = _build_nc()
    return _cache["nc"]


def _masks(g):
    """Additive mask for the last 256 columns of every strip.

    Strip for local tile k covers i in [0, (2k+2)*128); its last 256
    columns are i = 2k*128 + u, u in [0, 256). Partition p holds global
    j = (2k+g)*128 + p, so live (i <= j) iff u <= 128*g + p.
    """
    m = np.zeros((128, 256), np.float32)
    p = np.arange(128)[:, None]
    u = np.arange(256)[None, :]
    m[:] = np.where(u <= 128 * g + p, 0.0, _NEG)
    return m


def kernel(**inputs):
    from concourse.bass_utils import run_bass_kernel_spmd

    x = np.ascontiguousarray(np.asarray(inputs["x"], dtype=np.float32))
    Wq = np.ascontiguousarray(np.asarray(inputs["Wq"], dtype=np.float32))
    Wk = np.ascontiguousarray(np.asarray(inputs["Wk"], dtype=np.float32))
    Wv = np.asarray(inputs["Wv"], dtype=np.float32)
    bq = np.ascontiguousarray(
        np.asarray(inputs["bq"], dtype=np.float32).reshape(_K, 1))
    bk = np.ascontiguousarray(
        np.asarray(inputs["bk"], dtype=np.float32).reshape(_K, 1))
    bv = np.asarray(inputs["bv"], dtype=np.float32).ravel()

    import ml_dtypes
    bf = ml_dtypes.bfloat16
    x_bf = x.astype(bf)

    nc = _get_nc()
    in_maps = []
    for core in range(8):
        b, g = divmod(core, 2)
        # this core's j columns: global tiles {2k+g}, i.e. starts 256k+128g
        cols = ((np.arange(_JT) * 256 + 128 * g)[:, None]
                + np.arange(128)[None, :]).ravel()
        c_b = np.zeros((_C, 512), np.float32)
        c_b[:, 0:_K] = Wq
        c_b[:, _K:2 * _K] = Wk
        c_b[:, 128:256] = Wv
        c_b[:, 256:512] = _masks(g)
        c_f = np.zeros((_C, 130), np.float32)
        c_f[:, 0:128] = bv[None, :]
        c_f[0:_K, 128] = bq.ravel()
        c_f[0:_K, 129] = bk.ravel()
        in_maps.append({
            "xb": np.ascontiguousarray(x_bf[b]),
            "xj": np.ascontiguousarray(x_bf[b][:, cols]),
            "cb": np.ascontiguousarray(c_b.astype(bf)),
            "cf": np.ascontiguousarray(c_f),
        })

    trace = bool(_cache.get("trace"))
    res = run_bass_kernel_spmd(nc, in_maps, core_ids=list(range(8)),
                               trace=trace)
    _cache["last_result"] = res

    parts = [r["out"] for r in res.results]
    out = np.empty((_B, _C + _V, _T), np.float32)
    for b in range(_B):
        out[b, :_C] = x[b]
        out[b, _C:] = parts[2 * b] + parts[2 * b + 1]
    return out
